# revision 26
# baseline (speedup 1.0000x reference)
"""Trainium2 Bass kernel for nn_PositionalEncoding_61151744360729.

out[b, s, n, :] = x[b, s, n, :] + ||x[b, s+1, n, :] - x[b, s, n, :]||_2
(with distance 0 at s = S-1).

Sharding: data-parallel on batch across 8 NeuronCores (64 batches/core).

I/O is bf16 (the tolerance gate is rel-l2; bf16 quantization costs ~3e-3),
halving HBM traffic vs f32. The host pre-gathers each core's shard into a
chunk-contiguous coordinate-planar layout [batch, half, chunk-block] where
each chunk block is [c, frame, node]: every DMA is one 3-dim AP whose
per-partition row is a single contiguous span (span size drives DMA
efficiency: 9.7KB spans measured 305 GB/s, 19.3KB spans ~400 GB/s).

The chunk schedule is asymmetric — [32, 96, 128, 128, 96, 32] frames —
so the first compute starts after a ~5KB-span DMA instead of ~19KB
(shorter pipeline head) and the last output DMA is small (shorter tail).

Compute runs at half-chunk granularity for pipelining. All compute is on
DVE + ACT only:
  * Pool (GpSimd) is intentionally unused: measured traces show DVE ops
    slow down ~4x whenever a Pool op is in flight (shared-path
    contention), which costs DVE more than Pool contributes.
  * Every DVE operand is contiguous in its innermost free dim with bf16
    dtype — the DVE 2x perf-mode requirement. The dist broadcast over
    the 3 coords is three separate contiguous plane adds (a stride-0
    broadcast operand measurably drops DVE to 1x).
  * ACT does square and sqrt in place (no dtype speedup exists on ACT).
  * Sync issues all DMA (HWDGE), keeping it off the compute engines.
"""

import sys
from contextlib import ExitStack

for _p in ("/opt/trn_rl_repo", "/root/.axon_site/_ro/trn_rl_repo"):
    if _p not in sys.path:
        sys.path.insert(0, _p)

import numpy as np
import ml_dtypes

import concourse.bass as bass
import concourse.tile as tile
from concourse import bacc, mybir
from concourse.bass_utils import run_bass_kernel_spmd

BF16 = ml_dtypes.bfloat16

B, S, N, C = 512, 1024, 25, 3
NCORES = 8
BC = B // NCORES           # 64 batches per core
H = 2                      # sequence halves -> 128 partitions
SH = S // H                # 512 frames per half
P = H * BC                 # 128 partitions

CHUNKS = [32, 96, 128, 128, 112, 16]     # frames per chunk (sum = SH)
assert sum(CHUNKS) == SH and all(f % 2 == 0 for f in CHUNKS)
STARTS = [sum(CHUNKS[:i]) for i in range(len(CHUNKS))]
IN_CHUNKS = [C * (f + 1) * N for f in CHUNKS]   # +1 overlap frame
OUT_CHUNKS = [C * f * N for f in CHUNKS]
IN_OFFS = [sum(IN_CHUNKS[:i]) for i in range(len(CHUNKS))]
OUT_OFFS = [sum(OUT_CHUNKS[:i]) for i in range(len(CHUNKS))]
IN_TOT = sum(IN_CHUNKS)
OUT_TOT = sum(OUT_CHUNKS)                # = C * SH * N
IN_FLAT = BC * H * IN_TOT
OUT_FLAT = BC * H * OUT_TOT

_cache = {}


def _build():
    bf = mybir.dt.bfloat16
    f32 = mybir.dt.float32
    Af = mybir.ActivationFunctionType
    nc = bacc.Bacc(
        "TRN2", target_bir_lowering=False, debug=False, num_devices=NCORES
    )
    xin = nc.dram_tensor("xin", [IN_FLAT], bf, kind="ExternalInput")
    wid = nc.dram_tensor("wid", [128 * 256], bf, kind="ExternalInput")
    yout = nc.dram_tensor("yout", [OUT_FLAT], bf, kind="ExternalOutput")

    with tile.TileContext(nc) as tc, ExitStack() as ctx:
        pw = ctx.enter_context(tc.tile_pool(name="pw", bufs=1))
        pin = ctx.enter_context(tc.tile_pool(name="pin", bufs=4))
        pdiff = ctx.enter_context(tc.tile_pool(name="pdiff", bufs=4))
        pdist = ctx.enter_context(tc.tile_pool(name="pdist", bufs=4))
        pout = ctx.enter_context(tc.tile_pool(name="pout", bufs=3))
        ppsum = ctx.enter_context(tc.psum_pool(name="ppsum", bufs=2))

        PF = 3  # input prefetch depth (chunks)
        K = len(CHUNKS)

        # matmul moving windows: starts must sit on 512-elem (2KB) PSUM
        # bank boundaries — an accumulation-group reset (start=True) is
        # bank-granular, so a group must never straddle a bank.
        def mm_wins(fhn):
            wins = []
            w = 0
            while w < fhn:
                mw = min(512, fhn - w)
                wins.append((w, mw))
                w += mw
            return wins

        # +I / -I stationary weights for the PE shifted-subtract
        w_t = pw.tile([P, 256], bf)
        nc.sync.dma_start(
            w_t[:], bass.AP(wid, 0, [[256, 128], [1, 256]])
        )

        def issue_in(k):
            t = pin.tile([P, IN_CHUNKS[k]], bf)
            src = bass.AP(
                xin,
                IN_OFFS[k],
                [[H * IN_TOT, BC], [IN_TOT, H], [1, IN_CHUNKS[k]]],
            )
            nc.sync.dma_start(t[:], src)
            return t

        in_tiles = [issue_in(k) for k in range(PF)]

        for k in range(K):
            F = CHUNKS[k]
            FI = F + 1
            FH = F // 2
            FHN = FH * N

            in_t = in_tiles[k]
            t4 = in_t[:].rearrange("p (c f n) -> p c f n", c=C, f=FI, n=N)

            if k + PF < K:
                in_tiles.append(issue_in(k + PF))

            out_t = pout.tile([P, OUT_CHUNKS[k]], bf)
            o4 = out_t[:].rearrange("p (c f n) -> p c f n", c=C, f=F, n=N)

            # big chunks: shifted subtract on the (otherwise idle) PE via
            # paired +I/-I matmuls into PSUM; ACT squares straight from
            # PSUM. Keeps ~15us of subtracts off the DVE critical path.
            pe_sub = F >= 112

            for hv in range(2):
                f0 = hv * FH
                diff_t = pdiff.tile([P, C * FHN], bf)
                if pe_sub:
                    for c in range(C):
                        ps = ppsum.tile([P, 2048], f32)
                        nb = c * FI * N + (f0 + 1) * N
                        pb = c * FI * N + f0 * N
                        for w, mw in mm_wins(FHN):
                            nc.tensor.matmul(
                                ps[:, w: w + mw],
                                w_t[:, 0:128],
                                in_t[:, nb + w: nb + w + mw],
                                start=True,
                                stop=False,
                            )
                            nc.tensor.matmul(
                                ps[:, w: w + mw],
                                w_t[:, 128:256],
                                in_t[:, pb + w: pb + w + mw],
                                start=False,
                                stop=True,
                            )
                        nc.scalar.activation(
                            diff_t[:, c * FHN:(c + 1) * FHN],
                            ps[:, 0:FHN],
                            Af.Square,
                        )
                else:
                    d4 = diff_t[:].rearrange(
                        "p (c f n) -> p c f n", c=C, f=FH, n=N
                    )
                    nc.vector.tensor_sub(
                        d4,
                        t4[:, :, f0 + 1: f0 + FH + 1, :],
                        t4[:, :, f0: f0 + FH, :],
                    )
                    # square in place; diff_t becomes sq
                    nc.scalar.activation(diff_t[:], diff_t[:], Af.Square)
                s3 = diff_t[:].rearrange("p (c fn) -> p c fn", c=C, fn=FHN)

                dist_t = pdist.tile([P, FHN], bf)
                nc.vector.tensor_add(dist_t[:], s3[:, 0], s3[:, 1])
                nc.vector.tensor_add(dist_t[:], dist_t[:], s3[:, 2])
                # sqrt in place; dist_t becomes dist
                nc.scalar.activation(dist_t[:], dist_t[:], Af.Sqrt)

                dv = dist_t[:].rearrange("p (f n) -> p f n", f=FH, n=N)
                for c in range(C):
                    nc.vector.tensor_add(
                        o4[:, c, f0: f0 + FH, :],
                        t4[:, c, f0: f0 + FH, :],
                        dv,
                    )

            dst = bass.AP(
                yout,
                OUT_OFFS[k],
                [[H * OUT_TOT, BC], [OUT_TOT, H], [1, OUT_CHUNKS[k]]],
            )
            nc.sync.dma_start(dst, out_t[:])

    nc.compile()
    return nc


def kernel(x: np.ndarray, **_unused) -> np.ndarray:
    x = np.asarray(x)
    assert x.shape == (B, S, N, C), x.shape

    if "nc" not in _cache:
        _cache["nc"] = _build()
    nc = _cache["nc"]

    # f32 -> bf16, planar [B, C, S+1, N] with pad frame = last frame
    xb = x.astype(BF16).view(np.uint16)
    xt = np.ascontiguousarray(xb.transpose(0, 3, 1, 2))  # [B, C, S, N]
    xpad = np.concatenate([xt, xt[:, :, -1:, :]], axis=2)  # [B, C, S+1, N]

    # gather chunk frames into [B, H, IN_TOT] (chunk blocks are [C, FI, N])
    idx = np.concatenate(
        [
            (h * SH + STARTS[k] + np.arange(CHUNKS[k] + 1))
            for h in range(H)
            for k in range(len(CHUNKS))
        ]
    )  # frame indices incl. +1 overlap; max = S -> pad frame only at end
    # xpad[:, :, idx, :]: [B, C, H*IN_TOT/(C*N)... build per (h,k) blocks
    xg = np.empty((B, H, IN_TOT), dtype=np.uint16)
    pos = 0
    for h in range(H):
        for k in range(len(CHUNKS)):
            fi = CHUNKS[k] + 1
            fr = h * SH + STARTS[k]
            blk = xpad[:, :, fr: fr + fi, :]          # [B, C, fi, N]
            xg[:, h, IN_OFFS[k]: IN_OFFS[k] + C * fi * N] = blk.reshape(
                B, C * fi * N
            )
            pos += 1
    del idx

    wid = np.concatenate(
        [np.eye(128, dtype=np.float32), -np.eye(128, dtype=np.float32)],
        axis=1,
    ).astype(BF16)
    in_maps = [
        {
            "xin": xg[ci * BC:(ci + 1) * BC].reshape(IN_FLAT).view(BF16),
            "wid": wid.reshape(128 * 256),
        }
        for ci in range(NCORES)
    ]

    res = run_bass_kernel_spmd(nc, in_maps, core_ids=list(range(NCORES)))
    _cache["last_results"] = res

    out = np.empty((B, S, N, C), dtype=np.float32)
    for ci in range(NCORES):
        yc = np.asarray(res.results[ci]["yout"]).view(np.uint16)
        yc = yc.reshape(BC, H, OUT_TOT)
        yplanar = np.empty((BC, H, C, SH, N), dtype=np.uint16)
        for k in range(len(CHUNKS)):
            f = CHUNKS[k]
            blk = yc[:, :, OUT_OFFS[k]: OUT_OFFS[k] + C * f * N]
            yplanar[:, :, :, STARTS[k]: STARTS[k] + f, :] = blk.reshape(
                BC, H, C, f, N
            )
        # [BC, H, C, SH, N] -> [BC, (H SH)=S, N, C]
        yv = yplanar.transpose(0, 1, 3, 4, 2).reshape(BC, S, N, C)
        out[ci * BC:(ci + 1) * BC] = yv.view(BF16).astype(np.float32)
    return out


# revision 27
# speedup vs baseline: 1.0200x; 1.0200x over previous
"""Trainium2 Bass kernel for nn_PositionalEncoding_61151744360729.

out[b, s, n, :] = x[b, s, n, :] + ||x[b, s+1, n, :] - x[b, s, n, :]||_2
(with distance 0 at s = S-1).

Sharding: data-parallel on batch across 8 NeuronCores (64 batches/core).

I/O is bf16 (the tolerance gate is rel-l2; bf16 quantization costs ~3e-3),
halving HBM traffic vs f32. The host pre-gathers each core's shard into a
chunk-contiguous coordinate-planar layout [batch, half, chunk-block] where
each chunk block is [c, frame, node]: every DMA is one 3-dim AP whose
per-partition row is a single contiguous span (span size drives DMA
efficiency: 9.7KB spans measured 305 GB/s, 19.3KB spans ~400 GB/s).

The chunk schedule is asymmetric — [32, 96, 128, 128, 96, 32] frames —
so the first compute starts after a ~5KB-span DMA instead of ~19KB
(shorter pipeline head) and the last output DMA is small (shorter tail).

Compute runs at half-chunk granularity for pipelining. All compute is on
DVE + ACT only:
  * Pool (GpSimd) is intentionally unused: measured traces show DVE ops
    slow down ~4x whenever a Pool op is in flight (shared-path
    contention), which costs DVE more than Pool contributes.
  * Every DVE operand is contiguous in its innermost free dim with bf16
    dtype — the DVE 2x perf-mode requirement. The dist broadcast over
    the 3 coords is three separate contiguous plane adds (a stride-0
    broadcast operand measurably drops DVE to 1x).
  * ACT does square and sqrt in place (no dtype speedup exists on ACT).
  * Sync issues all DMA (HWDGE), keeping it off the compute engines.
"""

import sys
from contextlib import ExitStack

for _p in ("/opt/trn_rl_repo", "/root/.axon_site/_ro/trn_rl_repo"):
    if _p not in sys.path:
        sys.path.insert(0, _p)

import numpy as np
import ml_dtypes

import concourse.bass as bass
import concourse.tile as tile
from concourse import bacc, mybir
from concourse.bass_utils import run_bass_kernel_spmd

BF16 = ml_dtypes.bfloat16

B, S, N, C = 512, 1024, 25, 3
NCORES = 8
BC = B // NCORES           # 64 batches per core
H = 2                      # sequence halves -> 128 partitions
SH = S // H                # 512 frames per half
P = H * BC                 # 128 partitions

CHUNKS = [32, 96, 128, 128, 112, 16]     # frames per chunk (sum = SH)
assert sum(CHUNKS) == SH and all(f % 2 == 0 for f in CHUNKS)
STARTS = [sum(CHUNKS[:i]) for i in range(len(CHUNKS))]
IN_CHUNKS = [C * (f + 1) * N for f in CHUNKS]   # +1 overlap frame
OUT_CHUNKS = [C * f * N for f in CHUNKS]
IN_OFFS = [sum(IN_CHUNKS[:i]) for i in range(len(CHUNKS))]
OUT_OFFS = [sum(OUT_CHUNKS[:i]) for i in range(len(CHUNKS))]
IN_TOT = sum(IN_CHUNKS)
OUT_TOT = sum(OUT_CHUNKS)                # = C * SH * N
IN_FLAT = BC * H * IN_TOT
OUT_FLAT = BC * H * OUT_TOT

_cache = {}


def _build():
    bf = mybir.dt.bfloat16
    f32 = mybir.dt.float32
    Af = mybir.ActivationFunctionType
    nc = bacc.Bacc(
        "TRN2", target_bir_lowering=False, debug=False, num_devices=NCORES
    )
    xin = nc.dram_tensor("xin", [IN_FLAT], bf, kind="ExternalInput")
    wid = nc.dram_tensor("wid", [128 * 256], bf, kind="ExternalInput")
    yout = nc.dram_tensor("yout", [OUT_FLAT], bf, kind="ExternalOutput")

    with tile.TileContext(nc) as tc, ExitStack() as ctx:
        pw = ctx.enter_context(tc.tile_pool(name="pw", bufs=1))
        pin = ctx.enter_context(tc.tile_pool(name="pin", bufs=4))
        pdiff = ctx.enter_context(tc.tile_pool(name="pdiff", bufs=4))
        pdist = ctx.enter_context(tc.tile_pool(name="pdist", bufs=4))
        pout = ctx.enter_context(tc.tile_pool(name="pout", bufs=3))
        ppsum = ctx.enter_context(tc.psum_pool(name="ppsum", bufs=2))

        PF = 3  # input prefetch depth (chunks)
        K = len(CHUNKS)

        # matmul moving windows: starts must sit on 512-elem (2KB) PSUM
        # bank boundaries — an accumulation-group reset (start=True) is
        # bank-granular, so a group must never straddle a bank.
        def mm_wins(fhn):
            wins = []
            w = 0
            while w < fhn:
                mw = min(512, fhn - w)
                wins.append((w, mw))
                w += mw
            return wins

        # +I / -I stationary weights for the PE shifted-subtract
        w_t = pw.tile([P, 256], bf)
        nc.sync.dma_start(
            w_t[:], bass.AP(wid, 0, [[256, 128], [1, 256]])
        )

        def issue_in(k):
            t = pin.tile([P, IN_CHUNKS[k]], bf)
            src = bass.AP(
                xin,
                IN_OFFS[k],
                [[H * IN_TOT, BC], [IN_TOT, H], [1, IN_CHUNKS[k]]],
            )
            nc.sync.dma_start(t[:], src)
            return t

        in_tiles = [issue_in(k) for k in range(PF)]

        for k in range(K):
            F = CHUNKS[k]
            FI = F + 1
            FH = F // 2
            FHN = FH * N

            in_t = in_tiles[k]
            t4 = in_t[:].rearrange("p (c f n) -> p c f n", c=C, f=FI, n=N)

            if k + PF < K:
                in_tiles.append(issue_in(k + PF))

            out_t = pout.tile([P, OUT_CHUNKS[k]], bf)
            o4 = out_t[:].rearrange("p (c f n) -> p c f n", c=C, f=F, n=N)

            # big chunks: shifted subtract on the (otherwise idle) PE via
            # paired +I/-I matmuls into PSUM; ACT squares straight from
            # PSUM. Keeps ~10us of subtracts off the DVE critical path.
            pe_sub = F == 128

            for hv in range(2):
                f0 = hv * FH
                diff_t = pdiff.tile([P, C * FHN], bf)
                if pe_sub:
                    for c in range(C):
                        ps = ppsum.tile([P, 2048], f32)
                        nb = c * FI * N + (f0 + 1) * N
                        pb = c * FI * N + f0 * N
                        for w, mw in mm_wins(FHN):
                            nc.tensor.matmul(
                                ps[:, w: w + mw],
                                w_t[:, 0:128],
                                in_t[:, nb + w: nb + w + mw],
                                start=True,
                                stop=False,
                            )
                            nc.tensor.matmul(
                                ps[:, w: w + mw],
                                w_t[:, 128:256],
                                in_t[:, pb + w: pb + w + mw],
                                start=False,
                                stop=True,
                            )
                        nc.scalar.activation(
                            diff_t[:, c * FHN:(c + 1) * FHN],
                            ps[:, 0:FHN],
                            Af.Square,
                        )
                else:
                    d4 = diff_t[:].rearrange(
                        "p (c f n) -> p c f n", c=C, f=FH, n=N
                    )
                    nc.vector.tensor_sub(
                        d4,
                        t4[:, :, f0 + 1: f0 + FH + 1, :],
                        t4[:, :, f0: f0 + FH, :],
                    )
                    # square in place; diff_t becomes sq
                    nc.scalar.activation(diff_t[:], diff_t[:], Af.Square)
                s3 = diff_t[:].rearrange("p (c fn) -> p c fn", c=C, fn=FHN)

                dist_t = pdist.tile([P, FHN], bf)
                nc.vector.tensor_add(dist_t[:], s3[:, 0], s3[:, 1])
                nc.vector.tensor_add(dist_t[:], dist_t[:], s3[:, 2])
                # sqrt in place; dist_t becomes dist
                nc.scalar.activation(dist_t[:], dist_t[:], Af.Sqrt)

                dv = dist_t[:].rearrange("p (f n) -> p f n", f=FH, n=N)
                for c in range(C):
                    nc.vector.tensor_add(
                        o4[:, c, f0: f0 + FH, :],
                        t4[:, c, f0: f0 + FH, :],
                        dv,
                    )

            dst = bass.AP(
                yout,
                OUT_OFFS[k],
                [[H * OUT_TOT, BC], [OUT_TOT, H], [1, OUT_CHUNKS[k]]],
            )
            nc.sync.dma_start(dst, out_t[:])

    nc.compile()
    return nc


def kernel(x: np.ndarray, **_unused) -> np.ndarray:
    x = np.asarray(x)
    assert x.shape == (B, S, N, C), x.shape

    if "nc" not in _cache:
        _cache["nc"] = _build()
    nc = _cache["nc"]

    # f32 -> bf16, planar [B, C, S+1, N] with pad frame = last frame
    xb = x.astype(BF16).view(np.uint16)
    xt = np.ascontiguousarray(xb.transpose(0, 3, 1, 2))  # [B, C, S, N]
    xpad = np.concatenate([xt, xt[:, :, -1:, :]], axis=2)  # [B, C, S+1, N]

    # gather chunk frames into [B, H, IN_TOT] (chunk blocks are [C, FI, N])
    idx = np.concatenate(
        [
            (h * SH + STARTS[k] + np.arange(CHUNKS[k] + 1))
            for h in range(H)
            for k in range(len(CHUNKS))
        ]
    )  # frame indices incl. +1 overlap; max = S -> pad frame only at end
    # xpad[:, :, idx, :]: [B, C, H*IN_TOT/(C*N)... build per (h,k) blocks
    xg = np.empty((B, H, IN_TOT), dtype=np.uint16)
    pos = 0
    for h in range(H):
        for k in range(len(CHUNKS)):
            fi = CHUNKS[k] + 1
            fr = h * SH + STARTS[k]
            blk = xpad[:, :, fr: fr + fi, :]          # [B, C, fi, N]
            xg[:, h, IN_OFFS[k]: IN_OFFS[k] + C * fi * N] = blk.reshape(
                B, C * fi * N
            )
            pos += 1
    del idx

    wid = np.concatenate(
        [np.eye(128, dtype=np.float32), -np.eye(128, dtype=np.float32)],
        axis=1,
    ).astype(BF16)
    in_maps = [
        {
            "xin": xg[ci * BC:(ci + 1) * BC].reshape(IN_FLAT).view(BF16),
            "wid": wid.reshape(128 * 256),
        }
        for ci in range(NCORES)
    ]

    res = run_bass_kernel_spmd(nc, in_maps, core_ids=list(range(NCORES)))
    _cache["last_results"] = res

    out = np.empty((B, S, N, C), dtype=np.float32)
    for ci in range(NCORES):
        yc = np.asarray(res.results[ci]["yout"]).view(np.uint16)
        yc = yc.reshape(BC, H, OUT_TOT)
        yplanar = np.empty((BC, H, C, SH, N), dtype=np.uint16)
        for k in range(len(CHUNKS)):
            f = CHUNKS[k]
            blk = yc[:, :, OUT_OFFS[k]: OUT_OFFS[k] + C * f * N]
            yplanar[:, :, :, STARTS[k]: STARTS[k] + f, :] = blk.reshape(
                BC, H, C, f, N
            )
        # [BC, H, C, SH, N] -> [BC, (H SH)=S, N, C]
        yv = yplanar.transpose(0, 1, 3, 4, 2).reshape(BC, S, N, C)
        out[ci * BC:(ci + 1) * BC] = yv.view(BF16).astype(np.float32)
    return out
